# revision 25
# baseline (speedup 1.0000x reference)
"""Trainium2 Bass kernel for nn_Attention_cross (4-branch cross attention).

Data-parallel over batch: 16 samples -> 8 cores x 2 samples.
Streaming matmuls in bf16 (fp32 PSUM accumulate); tiny reduction matmuls fp32.

Per (sample, branch):
  qT   [HW,C]  = wq @ text^T                          (direct, transposed layout)
  kpre [C,HW]  = mk^T.T @ x ; vpre likewise
  k    [C,HW]  = grouped 3x3 conv == 9 block-diag matmuls on padded kpre
  k l2norm rows; k^T via 24 PE transposes (bf16)
  attn [C,C]   = qT.T @ kT, scaled by rq (row) post-hoc; instnorm+softmax
                 (instnorm mean cancels in softmax; only 1/std temperature kept)
  paT  [C,C]   = attn.T @ po^T        (P_o folded before attn@v, no transpose)
  out  [C,HW]  = paT.T @ v

Loop structure: Q stage for both samples first, then branch-outer /
sample-inner so each branch's weights (pointwise mats + block-diag conv
weights) are DMA'd once. DMAs are consolidated into single descriptors per
logical tensor (the SWDGE per-descriptor cost on SyncE was a co-bottleneck
with PE).
"""

import sys
from contextlib import ExitStack

import numpy as np

sys.path.insert(0, "/opt/trn_rl_repo")

import ml_dtypes  # noqa: E402
import concourse.bass as bass  # noqa: E402
import concourse.bacc as bacc  # noqa: E402
import concourse.mybir as mybir  # noqa: E402
import concourse.tile as tile  # noqa: E402
from concourse.bass_utils import run_bass_kernel_spmd  # noqa: E402

F32 = mybir.dt.float32
BF16 = mybir.dt.bfloat16
AF = mybir.ActivationFunctionType
ALU = mybir.AluOpType
AX = mybir.AxisListType
NPBF = ml_dtypes.bfloat16

B, C, H, W = 16, 384, 32, 32
HW = H * W          # 1024
TEXT = 768
NCORES = 8
SPC = B // NCORES   # samples per core = 2
CT = C // 128       # 3 c-tiles
HT = HW // 128      # 8 hw-tiles
TT = TEXT // 128    # 6 text-tiles
EPS_IN = 1e-5
NTOT = float(C * C)
WM = 9 * 128        # 1152: one conv weight-set row block


def build_program():
    nc = bacc.Bacc("TRN2", target_bir_lowering=False, debug=False,
                   num_devices=NCORES)

    emb = [nc.dram_tensor(f"emb{i+1}", [SPC, 128, CT * HW], BF16,
                          kind="ExternalInput") for i in range(4)]
    textt = nc.dram_tensor("textt", [SPC, 128, TT * C], BF16, kind="ExternalInput")
    wqt = nc.dram_tensor("wqt", [128, TT * HW], BF16, kind="ExternalInput")
    bqd = nc.dram_tensor("bqd", [1, HW], BF16, kind="ExternalInput")
    wmats = nc.dram_tensor("wmats", [4, 128, 3 * CT * C], BF16,
                           kind="ExternalInput")      # mk.T | mv.T | po.T tiled
    bdk = nc.dram_tensor("bdk", [4, 128, CT * WM], BF16, kind="ExternalInput")
    bdv = nc.dram_tensor("bdv", [4, 128, CT * WM], BF16, kind="ExternalInput")
    identd = nc.dram_tensor("identd", [128, 128], BF16, kind="ExternalInput")
    outs = [nc.dram_tensor(f"out{i+1}", [SPC, 128, CT * HW], F32,
                           kind="ExternalOutput") for i in range(4)]

    with tile.TileContext(nc) as tc, ExitStack() as ctx:
        const_p = ctx.enter_context(tc.tile_pool(name="const", bufs=1))
        wq_p = ctx.enter_context(tc.tile_pool(name="wq", bufs=1))
        tex_p = ctx.enter_context(tc.tile_pool(name="tex", bufs=2))
        mw_p = ctx.enter_context(tc.tile_pool(name="mw", bufs=2))
        bd_p = ctx.enter_context(tc.tile_pool(name="bd", bufs=2))
        x_p = ctx.enter_context(tc.tile_pool(name="x", bufs=2))
        qt_p = ctx.enter_context(tc.tile_pool(name="qt", bufs=2 * HT))
        qsq_p = ctx.enter_context(tc.tile_pool(name="qsq", bufs=2))
        rq_p = ctx.enter_context(tc.tile_pool(name="rq", bufs=4 * CT))
        pad_p = ctx.enter_context(tc.tile_pool(name="pad", bufs=6))
        k_p = ctx.enter_context(tc.tile_pool(name="k", bufs=CT))
        kt_p = ctx.enter_context(tc.tile_pool(name="kt", bufs=HT))
        v_p = ctx.enter_context(tc.tile_pool(name="v", bufs=2 * CT))
        at_p = ctx.enter_context(tc.tile_pool(name="at", bufs=2))
        sm_p = ctx.enter_context(tc.tile_pool(name="sm", bufs=8))
        o_p = ctx.enter_context(tc.tile_pool(name="o", bufs=2))
        pbig = ctx.enter_context(tc.tile_pool(name="pbig", bufs=5, space="PSUM"))
        pattn = ctx.enter_context(tc.tile_pool(name="pattn", bufs=2, space="PSUM"))
        ptr = ctx.enter_context(tc.tile_pool(name="ptr", bufs=1, space="PSUM"))
        ptiny = ptr

        # constants
        onesb = const_p.tile([128, 512], BF16)
        nc.vector.memset(onesb[:], 1.0)
        onesf = const_p.tile([128, 128], F32)
        nc.vector.memset(onesf[:], 1.0)
        ident = const_p.tile([128, 128], BF16)
        nc.sync.dma_start(ident[:], identd[:, :])
        bq = const_p.tile([1, HW], BF16)
        nc.sync.dma_start(bq[:], bqd[:, :])
        wq_sb = wq_p.tile([128, TT * HW], BF16)
        nc.sync.dma_start(wq_sb[:], wqt[:, :])

        # ================= Q stage (both samples) =================
        qT = [[] for _ in range(SPC)]
        rq = [[] for _ in range(SPC)]
        for s in range(SPC):
            tex_sb = tex_p.tile([128, TT * C], BF16, tag="tex")
            nc.sync.dma_start(tex_sb[:], textt[s, :, :])
            rqp = ptiny.tile([128, CT], F32, tag="tr", name="rqp",
                             padded_shape=[128, 96])
            for p in range(HT):
                ps = pattn.tile([128, C], F32, tag="pa", name="psa")
                for t in range(TT):
                    nc.tensor.matmul(
                        ps[:],
                        wq_sb[:, t * HW + p * 128: t * HW + (p + 1) * 128],
                        tex_sb[:, t * C:(t + 1) * C],
                        start=(t == 0), stop=False)
                nc.tensor.matmul(ps[:], bq[0:1, p * 128:(p + 1) * 128],
                                 onesb[0:1, 0:C], start=False, stop=True)
                qt = qt_p.tile([128, C], BF16, tag="qt")
                nc.vector.tensor_copy(qt[:], ps[:])
                qT[s].append(qt)
                qsq = qsq_p.tile([128, C], F32, tag="qsq")
                nc.scalar.activation(qsq[:], ps[:], AF.Square)
                for m in range(CT):
                    nc.tensor.matmul(
                        rqp[:, m:m + 1],
                        qsq[:, m * 128:(m + 1) * 128],
                        onesf[:, 0:1],
                        start=(p == 0), stop=(p == HT - 1))
            for m in range(CT):
                r_ = rq_p.tile([128, 1], F32, tag="rq")
                rt_ = rq_p.tile([128, 1], F32, tag="rqt")
                nc.scalar.activation(rt_[:], rqp[:, m:m + 1], AF.Sqrt, scale=float(C))
                nc.vector.reciprocal(r_[:], rt_[:])
                rq[s].append(r_)

        for i in range(4):
            wm_sb = mw_p.tile([128, 3 * CT * C], BF16, tag="wm")
            nc.sync.dma_start(wm_sb[:], wmats[i, :, :])
            mk_sb = wm_sb[:, 0:CT * C]
            mv_sb = wm_sb[:, CT * C:2 * CT * C]
            po_sb = wm_sb[:, 2 * CT * C:3 * CT * C]
            bdk_sb = bd_p.tile([128, CT * WM], BF16, tag="bdk")
            nc.sync.dma_start(bdk_sb[:], bdk[i, :, :])
            bdv_sb = bd_p.tile([128, CT * WM], BF16, tag="bdv")
            nc.sync.dma_start(bdv_sb[:], bdv[i, :, :])

            for s in range(SPC):
                x_sb = x_p.tile([128, CT * HW], BF16, tag="x")
                nc.sync.dma_start(x_sb[:], emb[i][s, :, :])

                def conv_branch(m_sb, bd_sb):
                    pads = []
                    for m in range(CT):
                        pad = pad_p.tile([128, 34, 34], BF16, tag="pad", name="pad")
                        nc.gpsimd.memset(pad[:, 0, :], 0.0)
                        nc.gpsimd.memset(pad[:, 33, :], 0.0)
                        nc.gpsimd.memset(pad[:, 1:33, 0], 0.0)
                        nc.gpsimd.memset(pad[:, 1:33, 33], 0.0)
                        for n in range(2):
                            ps = pbig.tile([128, 512], F32, tag="big", name="psb")
                            for t in range(CT):
                                nc.tensor.matmul(
                                    ps[:],
                                    m_sb[:, t * C + m * 128: t * C + (m + 1) * 128],
                                    x_sb[:, t * HW + n * 512: t * HW + n * 512 + 512],
                                    start=(t == 0), stop=(t == CT - 1))
                            h0 = n * 16
                            nc.vector.tensor_copy(
                                pad[:, 1 + h0:1 + h0 + 16, 1:33],
                                ps[:].rearrange("p (a b) -> p a b", a=16))
                        pads.append(pad)
                    chunks = []
                    for m in range(CT):
                        for n in range(2):
                            ps = pbig.tile([128, 512], F32, tag="big", name="psb")
                            h0 = n * 16
                            for dy in range(3):
                                for dx in range(3):
                                    d = dy * 3 + dx
                                    nc.tensor.matmul(
                                        ps[:],
                                        bd_sb[:, m * WM + d * 128:
                                              m * WM + (d + 1) * 128],
                                        pads[m][:, h0 + dy:h0 + dy + 16, dx:dx + 32],
                                        start=(d == 0), stop=(d == 8))
                            chunks.append(ps)
                    return chunks

                # ---- K path ----
                kch = conv_branch(mk_sb, bdk_sb)
                k_sb = []
                for m in range(CT):
                    ssq = sm_p.tile([128, 2], F32, tag="ksq")
                    junk = at_p.tile([128, 512], F32, tag="junk", bufs=1)
                    for n in range(2):
                        nc.scalar.activation(junk[:, 0:512], kch[2 * m + n][:],
                                             AF.Square, accum_out=ssq[:, n:n + 1])
                    tot = sm_p.tile([128, 1], F32, tag="ktot")
                    nc.vector.tensor_tensor(tot[:], ssq[:, 0:1], ssq[:, 1:2],
                                            op=ALU.add)
                    rk = sm_p.tile([128, 1], F32, tag="rk")
                    rkt = sm_p.tile([128, 1], F32, tag="rkt")
                    nc.scalar.activation(rkt[:], tot[:], AF.Sqrt)
                    nc.vector.reciprocal(rk[:], rkt[:])
                    kt_ = k_p.tile([128, HW], BF16, tag="k")
                    for n in range(2):
                        nc.vector.tensor_scalar_mul(
                            kt_[:, n * 512:(n + 1) * 512], kch[2 * m + n][:], rk[:])
                    k_sb.append(kt_)
                kT = []
                for p in range(HT):
                    pst = ptr.tile([128, C], BF16, tag="tr", name="pst")
                    for m in range(CT):
                        nc.tensor.transpose(
                            pst[:, m * 128:(m + 1) * 128],
                            k_sb[m][:, p * 128:(p + 1) * 128], ident[:])
                    ktile = kt_p.tile([128, C], BF16, tag="kt")
                    nc.scalar.activation(ktile[:], pst[:], AF.Copy)
                    kT.append(ktile)

                # ---- V path ----
                vch = conv_branch(mv_sb, bdv_sb)
                v_sb = []
                for m in range(CT):
                    vt = v_p.tile([128, HW], BF16, tag="v")
                    for n in range(2):
                        nc.vector.tensor_copy(vt[:, n * 512:(n + 1) * 512],
                                              vch[2 * m + n][:])
                    v_sb.append(vt)

                # ---- attention ----
                attn = []
                stats = sm_p.tile([128, 6], F32, tag="stats")
                for m in range(CT):
                    ps = pattn.tile([128, C], F32, tag="pa", name="psa")
                    for p in range(HT):
                        nc.tensor.matmul(
                            ps[:], qT[s][p][:, m * 128:(m + 1) * 128], kT[p][:],
                            start=(p == 0), stop=(p == HT - 1))
                    a_sb = at_p.tile([128, C], BF16, tag="attn", bufs=3)
                    nc.vector.tensor_scalar_mul(a_sb[:], ps[:], rq[s][m][:])
                    junk = at_p.tile([128, C], F32, tag="junk2", bufs=1)
                    nc.scalar.activation(junk[:], ps[:], AF.Square,
                                         scale=rq[s][m][:],
                                         accum_out=stats[:, 3 + m:4 + m])
                    nc.vector.tensor_reduce(stats[:, m:m + 1], a_sb[:],
                                            axis=AX.X, op=ALU.add)
                    attn.append(a_sb)
                stp = ptiny.tile([1, 6], F32, tag="tr", name="stp",
                                 padded_shape=[128, 96])
                nc.tensor.matmul(stp[:], onesf[:, 0:1], stats[:],
                                 start=True, stop=True)
                st_sb = sm_p.tile([1, 6], F32, tag="stsb")
                nc.vector.tensor_copy(st_sb[:], stp[:])
                S1 = sm_p.tile([1, 4], F32, tag="sca")
                nc.vector.tensor_reduce(S1[0:1, 0:1], st_sb[0:1, 0:3], axis=AX.X,
                                        op=ALU.add)
                nc.vector.tensor_reduce(S1[0:1, 1:2], st_sb[0:1, 3:6], axis=AX.X,
                                        op=ALU.add)
                nc.scalar.activation(S1[0:1, 2:3], S1[0:1, 0:1], AF.Copy,
                                     scale=1.0 / NTOT)
                nc.vector.tensor_tensor(S1[0:1, 3:4], S1[0:1, 2:3], S1[0:1, 2:3],
                                        op=ALU.mult)
                nc.scalar.activation(S1[0:1, 1:2], S1[0:1, 1:2], AF.Copy,
                                     scale=1.0 / NTOT)
                var = sm_p.tile([1, 1], F32, tag="var")
                nc.vector.tensor_tensor(var[:], S1[0:1, 1:2], S1[0:1, 3:4],
                                        op=ALU.subtract)
                vare = sm_p.tile([1, 1], F32, tag="vare")
                nc.vector.tensor_scalar_add(vare[:], var[:], EPS_IN)
                is_s = sm_p.tile([1, 1], F32, tag="iss")
                ist = sm_p.tile([1, 1], F32, tag="ist")
                nc.scalar.activation(ist[:], vare[:], AF.Sqrt)
                nc.vector.reciprocal(is_s[:], ist[:])
                isp = ptiny.tile([128, 1], F32, tag="tr", name="isp",
                                 padded_shape=[128, 96])
                nc.tensor.matmul(isp[:], onesf[0:1, 0:128], is_s[:],
                                 start=True, stop=True)
                is_col = sm_p.tile([128, 1], F32, tag="iscol")
                nc.scalar.activation(is_col[:], isp[:], AF.Copy)
                for m in range(CT):
                    mx = sm_p.tile([128, 1], F32, tag="mx")
                    nc.vector.tensor_reduce(mx[:], attn[m][:], axis=AX.X, op=ALU.max)
                    nb = sm_p.tile([128, 1], F32, tag="nb")
                    nc.vector.tensor_scalar(nb[:], mx[:], is_col[:], -1.0,
                                            op0=ALU.mult, op1=ALU.mult)
                    sume = sm_p.tile([128, 1], F32, tag="sume")
                    pex = at_p.tile([128, C], BF16, tag="pex", bufs=3)
                    nc.scalar.activation(pex[:], attn[m][:], AF.Exp, bias=nb[:],
                                         scale=is_col[:], accum_out=sume[:])
                    rcp = sm_p.tile([128, 1], F32, tag="rcp")
                    nc.vector.reciprocal(rcp[:], sume[:])
                    nc.vector.tensor_scalar_mul(attn[m][:], pex[:], rcp[:])
                # ---- paT = attn.T @ po_t ----
                paT = []
                for m in range(CT):
                    ps = pattn.tile([128, C], F32, tag="pa", name="psa")
                    for t in range(CT):
                        nc.tensor.matmul(
                            ps[:], attn[t][:, m * 128:(m + 1) * 128],
                            po_sb[:, t * C:(t + 1) * C],
                            start=(t == 0), stop=(t == CT - 1))
                    pa = at_p.tile([128, C], BF16, tag="pa", bufs=3, name="pa")
                    nc.vector.tensor_copy(pa[:], ps[:])
                    paT.append(pa)
                # ---- out = paT.T @ v ----
                o_full = o_p.tile([128, CT * HW], F32, tag="osb")
                for m in range(CT):
                    for n in range(2):
                        ps = pbig.tile([128, 512], F32, tag="big", name="psb")
                        for t in range(CT):
                            nc.tensor.matmul(
                                ps[:], paT[t][:, m * 128:(m + 1) * 128],
                                v_sb[t][:, n * 512:(n + 1) * 512],
                                start=(t == 0), stop=(t == CT - 1))
                        nc.scalar.activation(
                            o_full[:, m * HW + n * 512: m * HW + n * 512 + 512],
                            ps[:], AF.Copy)
                nc.sync.dma_start(outs[i][s, :, :], o_full[:])
    nc.compile()
    return nc


def _host_prep(inputs):
    emb_list = []
    for i in range(4):
        a = np.asarray(inputs[f"emb{i+1}"], np.float32).reshape(B, C, HW)
        t = a.reshape(B, CT, 128, HW).transpose(0, 2, 1, 3).reshape(B, 128, CT * HW)
        emb_list.append(np.ascontiguousarray(t).astype(NPBF))
    text = np.asarray(inputs["text_emb"], np.float32)
    wq = np.asarray(inputs["wq"], np.float32)
    bq = np.asarray(inputs["bq"], np.float32)
    mk, mv, po = (np.asarray(inputs[k], np.float32) for k in ("mk", "mv", "po"))
    kw, vw = (np.asarray(inputs[k], np.float32) for k in ("kw", "vw"))

    tt = text[:, 0].transpose(0, 2, 1).reshape(B, TT, 128, C)
    textt = np.ascontiguousarray(
        tt.transpose(0, 2, 1, 3).reshape(B, 128, TT * C)).astype(NPBF)
    wqt = np.ascontiguousarray(
        wq.T.reshape(TT, 128, HW).transpose(1, 0, 2).reshape(128, TT * HW)
    ).astype(NPBF)
    bqd = bq.reshape(1, HW).astype(NPBF)

    def wlayout(mat):  # [4,C,C] -> [4, 128, CT*C] of mat.T
        return (mat.transpose(0, 2, 1).reshape(4, CT, 128, C)
                .transpose(0, 2, 1, 3).reshape(4, 128, CT * C))
    wmats = np.ascontiguousarray(
        np.concatenate([wlayout(mk), wlayout(mv), wlayout(po)], axis=2)
    ).astype(NPBF)

    def bd(wt):
        out = np.zeros((4, CT, 128, 9, 128), np.float32)
        o = np.arange(C)
        g2 = (o // 2) * 2
        for j in range(2):
            cin = g2 + j
            for dy in range(3):
                for dx in range(3):
                    d = dy * 3 + dx
                    out[:, o // 128, cin % 128, d, o % 128] = wt[:, o, j, dy, dx]
        out = out.reshape(4, CT, 128, 9 * 128).transpose(0, 2, 1, 3)
        return np.ascontiguousarray(out.reshape(4, 128, CT * 9 * 128)).astype(NPBF)
    bdk_a, bdv_a = bd(kw), bd(vw)
    ident = np.eye(128, dtype=np.float32).astype(NPBF)

    in_maps = []
    for c in range(NCORES):
        sl = slice(c * SPC, (c + 1) * SPC)
        m = {"textt": textt[sl], "wqt": wqt, "bqd": bqd, "wmats": wmats,
             "bdk": bdk_a, "bdv": bdv_a, "identd": ident}
        for i in range(4):
            m[f"emb{i+1}"] = emb_list[i][sl]
        in_maps.append(m)
    return in_maps


_CACHED = {}


def kernel(**inputs):
    if "nc" not in _CACHED:
        _CACHED["nc"] = build_program()
    nc = _CACHED["nc"]
    in_maps = _host_prep(inputs)
    res = run_bass_kernel_spmd(nc, in_maps, list(range(NCORES)))
    outs = []
    for i in range(4):
        full = np.concatenate([res.results[c][f"out{i+1}"] for c in range(NCORES)],
                              axis=0)                       # [B, 128, CT*HW]
        full = full.reshape(B, 128, CT, HW).transpose(0, 2, 1, 3)
        outs.append(np.ascontiguousarray(
            full.reshape(B, C, H, W)).astype(np.float32))
    return tuple(outs)


if __name__ == "__main__":
    import reference
    inp = {k: np.asarray(v) for k, v in reference.setup_inputs().items()}
    got = kernel(**inp)
    exp = reference.reference(**inp)
    for i in range(4):
        e, g = np.asarray(exp[i]), got[i]
        err = np.abs(e - g).max() / (np.abs(e).max() + 1e-12)
        print(f"out{i+1}: rel err {err:.3e}")


# revision 27
# speedup vs baseline: 1.1374x; 1.1374x over previous
"""Trainium2 Bass kernel for nn_Attention_cross (4-branch cross attention).

Data-parallel over batch: 16 samples -> 8 cores x 2 samples.
Streaming matmuls in bf16 (fp32 PSUM accumulate); tiny reduction matmuls fp32.

Per (sample, branch):
  qT   [HW,C]  = wq @ text^T                          (direct, transposed layout)
  kpre [C,HW]  = mk^T.T @ x ; vpre likewise
  k    [C,HW]  = grouped 3x3 conv == 9 block-diag matmuls on padded kpre
  k l2norm rows; k^T via 24 PE transposes (bf16)
  attn [C,C]   = qT.T @ kT, scaled by rq (row) post-hoc; instnorm+softmax
                 (instnorm mean cancels in softmax; only 1/std temperature kept)
  paT  [C,C]   = attn.T @ po^T        (P_o folded before attn@v, no transpose)
  out  [C,HW]  = paT.T @ v

Loop structure: Q stage for both samples first, then branch-outer /
sample-inner so each branch's weights (pointwise mats + block-diag conv
weights) are DMA'd once. DMAs are consolidated into single descriptors per
logical tensor (the SWDGE per-descriptor cost on SyncE was a co-bottleneck
with PE).
"""

import sys
from contextlib import ExitStack

import numpy as np

sys.path.insert(0, "/opt/trn_rl_repo")

import ml_dtypes  # noqa: E402
import concourse.bass as bass  # noqa: E402
import concourse.bacc as bacc  # noqa: E402
import concourse.mybir as mybir  # noqa: E402
import concourse.tile as tile  # noqa: E402
from concourse.bass_utils import run_bass_kernel_spmd  # noqa: E402

F32 = mybir.dt.float32
BF16 = mybir.dt.bfloat16
AF = mybir.ActivationFunctionType
ALU = mybir.AluOpType
AX = mybir.AxisListType
NPBF = ml_dtypes.bfloat16

B, C, H, W = 16, 384, 32, 32
HW = H * W          # 1024
TEXT = 768
NCORES = 8
SPC = B // NCORES   # samples per core = 2
CT = C // 128       # 3 c-tiles
HT = HW // 128      # 8 hw-tiles
TT = TEXT // 128    # 6 text-tiles
EPS_IN = 1e-5
NTOT = float(C * C)
WM = 9 * 128        # 1152: one conv weight-set row block


def build_program():
    nc = bacc.Bacc("TRN2", target_bir_lowering=False, debug=False,
                   num_devices=NCORES)

    emb = [nc.dram_tensor(f"emb{i+1}", [SPC, 128, CT * HW], BF16,
                          kind="ExternalInput") for i in range(4)]
    textt = nc.dram_tensor("textt", [SPC, 128, TT * C], BF16, kind="ExternalInput")
    wqt = nc.dram_tensor("wqt", [128, TT * HW], BF16, kind="ExternalInput")
    bqd = nc.dram_tensor("bqd", [1, HW], BF16, kind="ExternalInput")
    wmats = nc.dram_tensor("wmats", [4, 128, 3 * CT * C], BF16,
                           kind="ExternalInput")      # mk.T | mv.T | po.T tiled
    bdk = nc.dram_tensor("bdk", [4, 128, CT * WM], BF16, kind="ExternalInput")
    bdv = nc.dram_tensor("bdv", [4, 128, CT * WM], BF16, kind="ExternalInput")
    identd = nc.dram_tensor("identd", [128, 128], BF16, kind="ExternalInput")
    outs = [nc.dram_tensor(f"out{i+1}", [SPC, 128, CT * HW], F32,
                           kind="ExternalOutput") for i in range(4)]

    with tile.TileContext(nc) as tc, ExitStack() as ctx:
        const_p = ctx.enter_context(tc.tile_pool(name="const", bufs=1))
        wq_p = ctx.enter_context(tc.tile_pool(name="wq", bufs=1))
        tex_p = ctx.enter_context(tc.tile_pool(name="tex", bufs=2))
        mw_p = ctx.enter_context(tc.tile_pool(name="mw", bufs=2))
        bd_p = ctx.enter_context(tc.tile_pool(name="bd", bufs=2))
        x_p = ctx.enter_context(tc.tile_pool(name="x", bufs=2))
        qt_p = ctx.enter_context(tc.tile_pool(name="qt", bufs=2 * HT))
        qsq_p = ctx.enter_context(tc.tile_pool(name="qsq", bufs=2))
        rq_p = ctx.enter_context(tc.tile_pool(name="rq", bufs=4 * CT))
        pad_p = ctx.enter_context(tc.tile_pool(name="pad", bufs=6))
        k_p = ctx.enter_context(tc.tile_pool(name="k", bufs=CT))
        kt_p = ctx.enter_context(tc.tile_pool(name="kt", bufs=HT))
        v_p = ctx.enter_context(tc.tile_pool(name="v", bufs=2 * CT))
        at_p = ctx.enter_context(tc.tile_pool(name="at", bufs=2))
        sm_p = ctx.enter_context(tc.tile_pool(name="sm", bufs=8))
        o_p = ctx.enter_context(tc.tile_pool(name="o", bufs=2))
        pbig = ctx.enter_context(tc.tile_pool(name="pbig", bufs=5, space="PSUM"))
        pattn = ctx.enter_context(tc.tile_pool(name="pattn", bufs=2, space="PSUM"))
        ptr = ctx.enter_context(tc.tile_pool(name="ptr", bufs=1, space="PSUM"))
        ptiny = ptr

        # constants
        onesb = const_p.tile([128, 512], BF16)
        nc.vector.memset(onesb[:], 1.0)
        onesf = const_p.tile([128, 128], F32)
        nc.vector.memset(onesf[:], 1.0)
        ident = const_p.tile([128, 128], BF16)
        nc.sync.dma_start(ident[:], identd[:, :])
        bq = const_p.tile([1, HW], BF16)
        nc.sync.dma_start(bq[:], bqd[:, :])
        wq_sb = wq_p.tile([128, TT * HW], BF16)
        nc.sync.dma_start(wq_sb[:], wqt[:, :])

        # ================= Q stage (both samples) =================
        qT = [[] for _ in range(SPC)]
        rq = [[] for _ in range(SPC)]
        for s in range(SPC):
            tex_sb = tex_p.tile([128, TT * C], BF16, tag="tex")
            nc.sync.dma_start(tex_sb[:], textt[s, :, :])
            rqp = ptiny.tile([128, CT], F32, tag="tr", name="rqp",
                             padded_shape=[128, 96])
            for p in range(HT):
                ps = pattn.tile([128, C], F32, tag="pa", name="psa")
                for t in range(TT):
                    nc.tensor.matmul(
                        ps[:],
                        wq_sb[:, t * HW + p * 128: t * HW + (p + 1) * 128],
                        tex_sb[:, t * C:(t + 1) * C],
                        start=(t == 0), stop=False)
                nc.tensor.matmul(ps[:], bq[0:1, p * 128:(p + 1) * 128],
                                 onesb[0:1, 0:C], start=False, stop=True)
                qt = qt_p.tile([128, C], BF16, tag="qt")
                nc.vector.tensor_copy(qt[:], ps[:])
                qT[s].append(qt)
                qsq = qsq_p.tile([128, C], F32, tag="qsq")
                nc.scalar.activation(qsq[:], ps[:], AF.Square)
                for m in range(CT):
                    nc.tensor.matmul(
                        rqp[:, m:m + 1],
                        qsq[:, m * 128:(m + 1) * 128],
                        onesf[:, 0:1],
                        start=(p == 0), stop=(p == HT - 1))
            for m in range(CT):
                r_ = rq_p.tile([128, 1], F32, tag="rq")
                rt_ = rq_p.tile([128, 1], F32, tag="rqt")
                nc.scalar.activation(rt_[:], rqp[:, m:m + 1], AF.Sqrt, scale=float(C))
                nc.vector.reciprocal(r_[:], rt_[:])
                rq[s].append(r_)

        for i in range(4):
            wm_sb = mw_p.tile([128, 3 * CT * C], BF16, tag="wm")
            nc.sync.dma_start(wm_sb[:], wmats[i, :, :])
            mk_sb = wm_sb[:, 0:CT * C]
            mv_sb = wm_sb[:, CT * C:2 * CT * C]
            po_sb = wm_sb[:, 2 * CT * C:3 * CT * C]
            bdk_sb = bd_p.tile([128, CT * WM], BF16, tag="bdk")
            nc.sync.dma_start(bdk_sb[:], bdk[i, :, :])
            bdv_sb = bd_p.tile([128, CT * WM], BF16, tag="bdv")
            nc.sync.dma_start(bdv_sb[:], bdv[i, :, :])

            for s in range(SPC):
                x_sb = x_p.tile([128, CT * HW], BF16, tag="x")
                nc.sync.dma_start(x_sb[:], emb[i][s, :, :])

                def conv_branch(m_sb, bd_sb):
                    pads = []
                    for m in range(CT):
                        pad = pad_p.tile([128, 34, 34], BF16, tag="pad", name="pad")
                        nc.gpsimd.memset(pad[:, 0, :], 0.0)
                        nc.gpsimd.memset(pad[:, 33, :], 0.0)
                        nc.gpsimd.memset(pad[:, 1:33, 0], 0.0)
                        nc.gpsimd.memset(pad[:, 1:33, 33], 0.0)
                        for n in range(2):
                            ps = pbig.tile([128, 512], F32, tag="big", name="psb")
                            for t in range(CT):
                                nc.tensor.matmul(
                                    ps[:],
                                    m_sb[:, t * C + m * 128: t * C + (m + 1) * 128],
                                    x_sb[:, t * HW + n * 512: t * HW + n * 512 + 512],
                                    start=(t == 0), stop=(t == CT - 1))
                            h0 = n * 16
                            nc.vector.tensor_copy(
                                pad[:, 1 + h0:1 + h0 + 16, 1:33],
                                ps[:].rearrange("p (a b) -> p a b", a=16))
                        pads.append(pad)
                    chunks = []
                    for m in range(CT):
                        for n in range(2):
                            ps = pbig.tile([128, 512], F32, tag="big", name="psb")
                            h0 = n * 16
                            for dy in range(3):
                                for dx in range(3):
                                    d = dy * 3 + dx
                                    nc.tensor.matmul(
                                        ps[:],
                                        bd_sb[:, m * WM + d * 128:
                                              m * WM + (d + 1) * 128],
                                        pads[m][:, h0 + dy:h0 + dy + 16, dx:dx + 32],
                                        start=(d == 0), stop=(d == 8))
                            chunks.append(ps)
                    return chunks

                # ---- K path ----
                kch = conv_branch(mk_sb, bdk_sb)
                k_sb = []
                for m in range(CT):
                    ssq = sm_p.tile([128, 2], F32, tag="ksq")
                    junk = at_p.tile([128, 512], F32, tag="junk", bufs=1)
                    for n in range(2):
                        nc.scalar.activation(junk[:, 0:512], kch[2 * m + n][:],
                                             AF.Square, accum_out=ssq[:, n:n + 1])
                    tot = sm_p.tile([128, 1], F32, tag="ktot")
                    nc.vector.tensor_tensor(tot[:], ssq[:, 0:1], ssq[:, 1:2],
                                            op=ALU.add)
                    rk = sm_p.tile([128, 1], F32, tag="rk")
                    rkt = sm_p.tile([128, 1], F32, tag="rkt")
                    nc.scalar.activation(rkt[:], tot[:], AF.Sqrt)
                    nc.vector.reciprocal(rk[:], rkt[:])
                    kt_ = k_p.tile([128, HW], BF16, tag="k")
                    for n in range(2):
                        nc.vector.tensor_scalar_mul(
                            kt_[:, n * 512:(n + 1) * 512], kch[2 * m + n][:], rk[:])
                    k_sb.append(kt_)
                kT = []
                for p in range(HT):
                    pst = ptr.tile([128, C], BF16, tag="tr", name="pst")
                    for m in range(CT):
                        nc.tensor.transpose(
                            pst[:, m * 128:(m + 1) * 128],
                            k_sb[m][:, p * 128:(p + 1) * 128], ident[:])
                    ktile = kt_p.tile([128, C], BF16, tag="kt")
                    nc.scalar.activation(ktile[:], pst[:], AF.Copy)
                    kT.append(ktile)

                # ---- V path ----
                vch = conv_branch(mv_sb, bdv_sb)
                v_sb = []
                for m in range(CT):
                    vt = v_p.tile([128, HW], BF16, tag="v")
                    for n in range(2):
                        nc.vector.tensor_copy(vt[:, n * 512:(n + 1) * 512],
                                              vch[2 * m + n][:])
                    v_sb.append(vt)

                # ---- attention ----
                attn = []
                stats = sm_p.tile([128, 6], F32, tag="stats")
                for m in range(CT):
                    ps = pattn.tile([128, C], F32, tag="pa", name="psa")
                    for p in range(HT):
                        nc.tensor.matmul(
                            ps[:], qT[s][p][:, m * 128:(m + 1) * 128], kT[p][:],
                            start=(p == 0), stop=(p == HT - 1))
                    a_sb = at_p.tile([128, C], BF16, tag="attn", bufs=3)
                    nc.vector.tensor_scalar_mul(a_sb[:], ps[:], rq[s][m][:])
                    junk = at_p.tile([128, C], F32, tag="junk2", bufs=1)
                    nc.scalar.activation(junk[:], ps[:], AF.Square,
                                         scale=rq[s][m][:],
                                         accum_out=stats[:, 3 + m:4 + m])
                    nc.vector.tensor_reduce(stats[:, m:m + 1], a_sb[:],
                                            axis=AX.X, op=ALU.add)
                    attn.append(a_sb)
                stp = ptiny.tile([1, 6], F32, tag="tr", name="stp",
                                 padded_shape=[128, 96])
                nc.tensor.matmul(stp[:], onesf[:, 0:1], stats[:],
                                 start=True, stop=True)
                st_sb = sm_p.tile([1, 6], F32, tag="stsb")
                nc.vector.tensor_copy(st_sb[:], stp[:])
                S1 = sm_p.tile([1, 4], F32, tag="sca")
                nc.vector.tensor_reduce(S1[0:1, 0:1], st_sb[0:1, 0:3], axis=AX.X,
                                        op=ALU.add)
                nc.vector.tensor_reduce(S1[0:1, 1:2], st_sb[0:1, 3:6], axis=AX.X,
                                        op=ALU.add)
                nc.scalar.activation(S1[0:1, 2:3], S1[0:1, 0:1], AF.Copy,
                                     scale=1.0 / NTOT)
                nc.vector.tensor_tensor(S1[0:1, 3:4], S1[0:1, 2:3], S1[0:1, 2:3],
                                        op=ALU.mult)
                nc.scalar.activation(S1[0:1, 1:2], S1[0:1, 1:2], AF.Copy,
                                     scale=1.0 / NTOT)
                var = sm_p.tile([1, 1], F32, tag="var")
                nc.vector.tensor_tensor(var[:], S1[0:1, 1:2], S1[0:1, 3:4],
                                        op=ALU.subtract)
                vare = sm_p.tile([1, 1], F32, tag="vare")
                nc.vector.tensor_scalar_add(vare[:], var[:], EPS_IN)
                is_s = sm_p.tile([1, 1], F32, tag="iss")
                ist = sm_p.tile([1, 1], F32, tag="ist")
                nc.scalar.activation(ist[:], vare[:], AF.Sqrt)
                nc.vector.reciprocal(is_s[:], ist[:])
                isp = ptiny.tile([128, 1], F32, tag="tr", name="isp",
                                 padded_shape=[128, 96])
                nc.tensor.matmul(isp[:], onesf[0:1, 0:128], is_s[:],
                                 start=True, stop=True)
                is_col = sm_p.tile([128, 1], F32, tag="iscol")
                nc.scalar.activation(is_col[:], isp[:], AF.Copy)
                for m in range(CT):
                    mx = sm_p.tile([128, 1], F32, tag="mx")
                    nc.vector.tensor_reduce(mx[:], attn[m][:], axis=AX.X, op=ALU.max)
                    nb = sm_p.tile([128, 1], F32, tag="nb")
                    nc.vector.tensor_scalar(nb[:], mx[:], is_col[:], -1.0,
                                            op0=ALU.mult, op1=ALU.mult)
                    sume = sm_p.tile([128, 1], F32, tag="sume")
                    pex = at_p.tile([128, C], BF16, tag="pex", bufs=3)
                    nc.scalar.activation(pex[:], attn[m][:], AF.Exp, bias=nb[:],
                                         scale=is_col[:], accum_out=sume[:])
                    rcp = sm_p.tile([128, 1], F32, tag="rcp")
                    nc.vector.reciprocal(rcp[:], sume[:])
                    nc.vector.tensor_scalar_mul(attn[m][:], pex[:], rcp[:])
                # ---- paT = attn.T @ po_t ----
                paT = []
                for m in range(CT):
                    ps = pattn.tile([128, C], F32, tag="pa", name="psa")
                    for t in range(CT):
                        nc.tensor.matmul(
                            ps[:], attn[t][:, m * 128:(m + 1) * 128],
                            po_sb[:, t * C:(t + 1) * C],
                            start=(t == 0), stop=(t == CT - 1))
                    pa = at_p.tile([128, C], BF16, tag="pa", bufs=3, name="pa")
                    nc.vector.tensor_copy(pa[:], ps[:])
                    paT.append(pa)
                # ---- out = paT.T @ v ----
                o_full = o_p.tile([128, CT * HW], F32, tag="osb")
                for m in range(CT):
                    for n in range(2):
                        ps = pattn.tile([128, 512], F32, tag="pa", name="psa")
                        for t in range(CT):
                            nc.tensor.matmul(
                                ps[:], paT[t][:, m * 128:(m + 1) * 128],
                                v_sb[t][:, n * 512:(n + 1) * 512],
                                start=(t == 0), stop=(t == CT - 1))
                        nc.scalar.activation(
                            o_full[:, m * HW + n * 512: m * HW + n * 512 + 512],
                            ps[:], AF.Copy)
                    nc.sync.dma_start(outs[i][s, :, m * HW:(m + 1) * HW],
                                      o_full[:, m * HW:(m + 1) * HW])
    nc.compile()
    return nc


def _host_prep(inputs):
    emb_list = []
    for i in range(4):
        a = np.asarray(inputs[f"emb{i+1}"], np.float32).reshape(B, C, HW)
        t = a.reshape(B, CT, 128, HW).transpose(0, 2, 1, 3).reshape(B, 128, CT * HW)
        emb_list.append(np.ascontiguousarray(t).astype(NPBF))
    text = np.asarray(inputs["text_emb"], np.float32)
    wq = np.asarray(inputs["wq"], np.float32)
    bq = np.asarray(inputs["bq"], np.float32)
    mk, mv, po = (np.asarray(inputs[k], np.float32) for k in ("mk", "mv", "po"))
    kw, vw = (np.asarray(inputs[k], np.float32) for k in ("kw", "vw"))

    tt = text[:, 0].transpose(0, 2, 1).reshape(B, TT, 128, C)
    textt = np.ascontiguousarray(
        tt.transpose(0, 2, 1, 3).reshape(B, 128, TT * C)).astype(NPBF)
    wqt = np.ascontiguousarray(
        wq.T.reshape(TT, 128, HW).transpose(1, 0, 2).reshape(128, TT * HW)
    ).astype(NPBF)
    bqd = bq.reshape(1, HW).astype(NPBF)

    def wlayout(mat):  # [4,C,C] -> [4, 128, CT*C] of mat.T
        return (mat.transpose(0, 2, 1).reshape(4, CT, 128, C)
                .transpose(0, 2, 1, 3).reshape(4, 128, CT * C))
    wmats = np.ascontiguousarray(
        np.concatenate([wlayout(mk), wlayout(mv), wlayout(po)], axis=2)
    ).astype(NPBF)

    def bd(wt):
        out = np.zeros((4, CT, 128, 9, 128), np.float32)
        o = np.arange(C)
        g2 = (o // 2) * 2
        for j in range(2):
            cin = g2 + j
            for dy in range(3):
                for dx in range(3):
                    d = dy * 3 + dx
                    out[:, o // 128, cin % 128, d, o % 128] = wt[:, o, j, dy, dx]
        out = out.reshape(4, CT, 128, 9 * 128).transpose(0, 2, 1, 3)
        return np.ascontiguousarray(out.reshape(4, 128, CT * 9 * 128)).astype(NPBF)
    bdk_a, bdv_a = bd(kw), bd(vw)
    ident = np.eye(128, dtype=np.float32).astype(NPBF)

    in_maps = []
    for c in range(NCORES):
        sl = slice(c * SPC, (c + 1) * SPC)
        m = {"textt": textt[sl], "wqt": wqt, "bqd": bqd, "wmats": wmats,
             "bdk": bdk_a, "bdv": bdv_a, "identd": ident}
        for i in range(4):
            m[f"emb{i+1}"] = emb_list[i][sl]
        in_maps.append(m)
    return in_maps


_CACHED = {}


def kernel(**inputs):
    if "nc" not in _CACHED:
        _CACHED["nc"] = build_program()
    nc = _CACHED["nc"]
    in_maps = _host_prep(inputs)
    res = run_bass_kernel_spmd(nc, in_maps, list(range(NCORES)))
    outs = []
    for i in range(4):
        full = np.concatenate([res.results[c][f"out{i+1}"] for c in range(NCORES)],
                              axis=0)                       # [B, 128, CT*HW]
        full = full.reshape(B, 128, CT, HW).transpose(0, 2, 1, 3)
        outs.append(np.ascontiguousarray(
            full.reshape(B, C, H, W)).astype(np.float32))
    return tuple(outs)


if __name__ == "__main__":
    import reference
    inp = {k: np.asarray(v) for k, v in reference.setup_inputs().items()}
    got = kernel(**inp)
    exp = reference.reference(**inp)
    for i in range(4):
        e, g = np.asarray(exp[i]), got[i]
        err = np.abs(e - g).max() / (np.abs(e).max() + 1e-12)
        print(f"out{i+1}: rel err {err:.3e}")
